# revision 1
# baseline (speedup 1.0000x reference)
"""Trainium2 Bass kernel for nn_Attention_57080115364834.

Reference computation (B=4, C=512, H=W=64, N=H*W=4096 tokens):
    t = x.reshape(b, c, n).swapaxes(1, 2)          # (b, n, c)
    q, k, v = t@Wq.T+bq, t@Wk.T+bk, t@Wv.T+bv
    attn = softmax(q @ k.T / sqrt(c))              # (b, n, n)
    out = (attn @ v) @ Wo.T + bo                   # (b, n, c)
    return out.reshape(b, c, h, w)                 # raw view, no permute

Sharding: 8 cores = 4 batches x 2 query-halves. Each core holds the full
x[b] (C x N, which is exactly t.T - the natural layout for Trainium
matmuls) so it computes its batch's full K^T (c,n) and VW (n,c) locally,
plus Q^T for its 2048-token half. No collectives.

Host-side algebra folds both post-attention linear steps away:
  - softmax rows sum to 1  =>  attn @ (v+bv) == attn@v + bv, so the v
    bias becomes an output bias  bo' = Wo @ bv + bo.
  - (attn @ v) @ Wo.T == attn @ (v @ Wo.T) == attn @ (t @ (Wo@Wv).T),
    so with Wvo = Wo@Wv precomputed on host, the VW projection directly
    produces final-channel values and no device-side output projection
    is needed.
The kernel returns outT (c, n) per core; the host transposes during
unsharding (a pure layout move).

Per-core dataflow (main matmuls bf16 with f32 PSUM accumulation; the
softmax normalization chain runs in f32/f32r, so 1/rowsum is exact):
  kT[c,m]   = Wk @ tC + bk   (lhsT=WkT chunk, rhs=tC chunk; bias on ACT evac)
  VW[m,c]   = tC.T @ WvoT    (lhsT=tC chunk,  rhs=WvoT)
  qT[c,n]   = Wq @ tCq + bq  per 512-token n-chunk
  ST[m,n]   = kT.T-chunks @ qT       (scores, transposed)
  P[m,n]    = exp(ST/sqrt(c))        ScalarE, no max-subtract (|scores|<~2)
  acc[m%128,n] += P                  DVE accumulate (for rowsum)
  OT[c,n]  += VW-chunk.T @ P         (PSUM-accumulated over m-tiles)
  OT[c,n]  += bo'[c-chunk] x rowsum[n]   (K=1 matmul; exact bias)
  rowsum[1,n] = ones.T @ acc (f32r MM); rinv broadcast via K=1 MM
  outT[c,n] = OT * rinv_bc           (DVE, PSUM->SBUF) -> DMA

Chunk tails are software-pipelined: chunk nb's rowsum/normalize/store is
emitted after chunk nb+1's q-projection so the scheduler never stalls
the TensorEngine on the rowsum chain at chunk boundaries.
"""

import sys

for _p in ("/opt/trn_rl_repo", "/root/.axon_site/_ro/trn_rl_repo"):
    if _p not in sys.path:
        sys.path.append(_p)

import numpy as np
import ml_dtypes

import concourse.bacc as bacc
import concourse.mybir as mybir
import concourse.tile as tile
from concourse.bass_utils import run_bass_kernel_spmd

DT = mybir.dt.float32
FR = mybir.dt.float32r
BF = mybir.dt.bfloat16
AFT = mybir.ActivationFunctionType

B, C, HW = 4, 512, 4096          # batch, channels, tokens per batch
NQ = HW // 2                     # q tokens per core (2048)
CK = C // 128                    # contraction chunks (4)
MT = HW // 128                   # key/value tiles (32)
NB = NQ // 512                   # q-chunks per core (4)
SCALE = 1.0 / float(np.sqrt(C))
N_CORES = 8

_compiled = None
_ONES = np.ones(128, dtype=np.float32)


def _build():
    nc = bacc.Bacc("TRN2", target_bir_lowering=False)

    xt_e = nc.declare_dram_parameter("xt", [C, HW], BF, isOutput=False)
    xq_e = nc.declare_dram_parameter("xq", [C, NQ], BF, isOutput=False)
    wqt_e = nc.declare_dram_parameter("wqt", [C, C], BF, isOutput=False)
    wkt_e = nc.declare_dram_parameter("wkt", [C, C], BF, isOutput=False)
    wvot_e = nc.declare_dram_parameter("wvot", [C, C], BF, isOutput=False)
    bq_e = nc.declare_dram_parameter("bq", [C], DT, isOutput=False)
    bk_e = nc.declare_dram_parameter("bk", [C], DT, isOutput=False)
    bop_e = nc.declare_dram_parameter("bop", [C], BF, isOutput=False)
    ones_fr_e = nc.declare_dram_parameter("ones_fr", [128], FR, isOutput=False)
    out_e = nc.declare_dram_parameter("outT", [C, NQ], DT, isOutput=True)

    with tile.TileContext(nc) as tc:
        with (
            tc.tile_pool(name="kt", bufs=1) as kt_pool,
            tc.tile_pool(name="vv", bufs=1) as vv_pool,
            tc.tile_pool(name="wq", bufs=1) as wq_pool,
            tc.tile_pool(name="consts", bufs=1) as c_pool,
        ):
            # ---- persistent tiles (phase-2-only DMAs emitted late so they
            # don't delay the first phase-1 matmul) ----
            kt_sb = [kt_pool.tile([128, HW], BF, tag=f"k{i}", name=f"k{i}") for i in range(CK)]
            vw_sb = [vv_pool.tile([128, C], BF, tag=f"v{i}", name=f"v{i}") for i in range(MT)]
            wq_sb = [wq_pool.tile([128, C], BF, tag=f"wq{i}", name=f"wq{i}") for i in range(CK)]

            bq_t = c_pool.tile([128, CK], DT, tag="bq", name="bq_t")
            bk_t = c_pool.tile([128, CK], DT, tag="bk", name="bk_t")
            bop_row = c_pool.tile([1, C], BF, tag="bop", name="bop_row")
            ones_col_r = c_pool.tile([128, 1], FR, tag="onescr", name="ones_col_r")
            ones_row_r = c_pool.tile([1, 128], FR, tag="onesrr", name="ones_row_r")
            for t in range(CK):
                nc.sync.dma_start(bk_t[:, t:t + 1], bk_e[t * 128:(t + 1) * 128])
            nc.sync.dma_start(ones_col_r[:, 0:1], ones_fr_e[:])
            nc.sync.dma_start(ones_row_r[0:1, :], ones_fr_e[:])

            # ---- phase 1: kT (c,m) and VW (m,c) projections ----
            with (
                tc.tile_pool(name="wkv", bufs=1) as wkv_pool,
                tc.tile_pool(name="tcc", bufs=3) as tcc_pool,
                tc.tile_pool(name="ps1", bufs=2, space="PSUM") as ps1,
            ):
                wk_sb = [wkv_pool.tile([128, C], BF, tag=f"wk{i}", name=f"wk{i}") for i in range(CK)]
                wv_sb = [wkv_pool.tile([128, C], BF, tag=f"wv{i}", name=f"wv{i}") for i in range(CK)]
                for i in range(CK):
                    nc.sync.dma_start(wk_sb[i][:], wkt_e[i * 128:(i + 1) * 128, :])
                for i in range(CK):
                    nc.sync.dma_start(wv_sb[i][:], wvot_e[i * 128:(i + 1) * 128, :])

                for j in range(HW // 512):
                    tcs = [tcc_pool.tile([128, 512], BF, tag=f"tc{ci}", name=f"tc{ci}") for ci in range(CK)]
                    for ci in range(CK):
                        nc.gpsimd.dma_start(
                            tcs[ci][:], xt_e[ci * 128:(ci + 1) * 128, j * 512:(j + 1) * 512]
                        )
                    # kT token-chunk j, all four output-channel chunks
                    for co in range(CK):
                        pk = ps1.tile([128, 512], DT, tag="pk", name="pk")
                        for ci in range(CK):
                            nc.tensor.matmul(
                                pk[:], wk_sb[ci][:, co * 128:(co + 1) * 128],
                                tcs[ci][:], start=(ci == 0), stop=(ci == CK - 1),
                            )
                        nc.scalar.activation(
                            kt_sb[co][:, j * 512:(j + 1) * 512], pk[:], AFT.Identity,
                            bias=bk_t[:, co:co + 1],
                        )
                    # VW m-tiles 4j..4j+3 (no bias: folded into bo')
                    for ml in range(4):
                        pv = ps1.tile([128, 512], DT, tag="pv", name="pv")
                        for ci in range(CK):
                            nc.tensor.matmul(
                                pv[:], tcs[ci][:, ml * 128:(ml + 1) * 128],
                                wv_sb[ci][:], start=(ci == 0), stop=(ci == CK - 1),
                            )
                        nc.vector.tensor_copy(vw_sb[4 * j + ml][:], pv[:])

            # phase-2 weights/consts arrive while phase-1 compute runs
            for i in range(CK):
                nc.sync.dma_start(wq_sb[i][:], wqt_e[i * 128:(i + 1) * 128, :])
            for t in range(CK):
                nc.sync.dma_start(bq_t[:, t:t + 1], bq_e[t * 128:(t + 1) * 128])
            nc.sync.dma_start(bop_row[0:1, :], bop_e[:])

            # ---- phase 2: attention per 512-token q-chunk ----
            with (
                tc.tile_pool(name="xqp", bufs=2) as xq_pool,
                tc.tile_pool(name="qcp", bufs=2) as qc_pool,
                tc.tile_pool(name="pexp", bufs=6) as pe_pool,
                tc.tile_pool(name="accp", bufs=2) as acc_pool,
                tc.tile_pool(name="rsp", bufs=2) as rs_pool,
                tc.tile_pool(name="outp", bufs=3) as out_pool,
                tc.tile_pool(name="smallp", bufs=2) as small_pool,
                tc.tile_pool(name="ps2", bufs=4, space="PSUM") as ps2,
                tc.tile_pool(name="psot", bufs=1, space="PSUM") as psot,
            ):
                def emit_tail(tnb, acc, ots):
                    # rowsum via one f32r ones-matmul; reciprocal row;
                    # broadcast via K=1 matmul; exact bias; normalize + store
                    rs = ps2.tile([1, 512], DT, tag="st", name="rs")
                    nc.tensor.matmul(rs[:], ones_col_r[:, 0:1], acc[:], start=True, stop=True)
                    rs_row = small_pool.tile([1, 512], BF, tag="rsrow", name="rs_row")
                    nc.scalar.activation(rs_row[:], rs[:], AFT.Copy)
                    rinv_row = small_pool.tile([1, 512], FR, tag="rinvrow", name="rinv_row")
                    with nc.allow_low_precision(reason="f32r stores full f32 bits; PE rounds on read"):
                        nc.vector.reciprocal(rinv_row[:], rs[:])
                    rbc_ps = ps2.tile([128, 512], DT, tag="st", name="rbc_ps")
                    nc.tensor.matmul(rbc_ps[:], ones_row_r[0:1, :], rinv_row[0:1, :],
                                     start=True, stop=True)
                    rinv_bc = rs_pool.tile([128, 512], DT, tag="rinvbc", name="rinv_bc")
                    nc.vector.tensor_copy(rinv_bc[:], rbc_ps[:])
                    for co in range(CK):
                        nc.tensor.matmul(
                            ots[co][:], bop_row[0:1, co * 128:(co + 1) * 128],
                            rs_row[0:1, :], start=False, stop=True, skip_group_check=True,
                        )
                        oc = out_pool.tile([128, 512], DT, tag="oc", name="oc", bufs=5)
                        nc.vector.tensor_mul(oc[:], ots[co][:], rinv_bc[:])
                        nc.sync.dma_start(
                            out_e[co * 128:(co + 1) * 128, tnb * 512:(tnb + 1) * 512], oc[:]
                        )

                prev = None
                for nb in range(NB):
                    xqs = [xq_pool.tile([128, 512], BF, tag=f"xq{ci}", name=f"xq{ci}") for ci in range(CK)]
                    for ci in range(CK):
                        nc.gpsimd.dma_start(
                            xqs[ci][:], xq_e[ci * 128:(ci + 1) * 128, nb * 512:(nb + 1) * 512]
                        )
                    # qT chunk (c, 512)
                    qcs = []
                    for co in range(CK):
                        pq = ps2.tile([128, 512], DT, tag="st", name="st")
                        for ci in range(CK):
                            nc.tensor.matmul(
                                pq[:], wq_sb[ci][:, co * 128:(co + 1) * 128],
                                xqs[ci][:], start=(ci == 0), stop=(ci == CK - 1),
                            )
                        qc = qc_pool.tile([128, 512], BF, tag=f"qc{co}", name=f"qc{co}")
                        nc.scalar.activation(qc[:], pq[:], AFT.Identity, bias=bq_t[:, co:co + 1])
                        qcs.append(qc)

                    # previous chunk's tail is emitted here so its rowsum chain
                    # never blocks this chunk's q-projection in the ACT queue
                    if prev is not None:
                        emit_tail(*prev)

                    acc = acc_pool.tile([128, 512], FR, tag="acc", name="acc")
                    ots = [psot.tile([128, 512], DT, tag=f"ot{co}", name=f"ot{co}") for co in range(CK)]
                    for mt in range(MT):
                        st = ps2.tile([128, 512], DT, tag="st", name="st")
                        for ci in range(CK):
                            nc.tensor.matmul(
                                st[:], kt_sb[ci][:, mt * 128:(mt + 1) * 128],
                                qcs[ci][:], start=(ci == 0), stop=(ci == CK - 1),
                            )
                        pexp = pe_pool.tile([128, 512], BF, tag="pe", name="pexp")
                        nc.scalar.activation(pexp[:], st[:], AFT.Exp, scale=SCALE)
                        if mt == 0:
                            nc.vector.tensor_copy(acc[:], pexp[:])
                        else:
                            nc.vector.tensor_add(acc[:], acc[:], pexp[:])
                        for co in range(CK):
                            nc.tensor.matmul(
                                ots[co][:], vw_sb[mt][:, co * 128:(co + 1) * 128],
                                pexp[:],
                                start=(mt == 0), stop=False, skip_group_check=True,
                            )
                    prev = (nb, acc, ots)

                emit_tail(*prev)

    nc.compile()
    return nc


def _get_compiled():
    global _compiled
    if _compiled is None:
        _compiled = _build()
    return _compiled


def kernel(**inputs):
    x = np.ascontiguousarray(np.asarray(inputs["x"], dtype=np.float32))
    wq = np.asarray(inputs["Wq"], dtype=np.float32)
    wk = np.asarray(inputs["Wk"], dtype=np.float32)
    wv = np.asarray(inputs["Wv"], dtype=np.float32)
    wo = np.asarray(inputs["Wo"], dtype=np.float32)
    bq = np.ascontiguousarray(np.asarray(inputs["bq"], dtype=np.float32))
    bk = np.ascontiguousarray(np.asarray(inputs["bk"], dtype=np.float32))
    bv = np.asarray(inputs["bv"], dtype=np.float32)
    bo = np.asarray(inputs["bo"], dtype=np.float32)

    wqt = np.ascontiguousarray(wq.T.astype(ml_dtypes.bfloat16))
    wkt = np.ascontiguousarray(wk.T.astype(ml_dtypes.bfloat16))
    wvot = np.ascontiguousarray((wo @ wv).T.astype(ml_dtypes.bfloat16))
    bop = np.ascontiguousarray((wo @ bv + bo).astype(ml_dtypes.bfloat16))

    xb = x.reshape(B, C, HW).astype(ml_dtypes.bfloat16)
    in_maps = []
    for core in range(N_CORES):
        bi, h = core // 2, core % 2
        in_maps.append({
            "xt": np.ascontiguousarray(xb[bi]),
            "xq": np.ascontiguousarray(xb[bi][:, h * NQ:(h + 1) * NQ]),
            "wqt": wqt, "wkt": wkt, "wvot": wvot,
            "bq": bq, "bk": bk, "bop": bop, "ones_fr": _ONES,
        })

    nc = _get_compiled()
    res = run_bass_kernel_spmd(nc, in_maps, core_ids=list(range(N_CORES)))

    out = np.empty((B, HW, C), dtype=np.float32)
    for core in range(N_CORES):
        bi, h = core // 2, core % 2
        out[bi, h * NQ:(h + 1) * NQ, :] = res.results[core]["outT"].T
    return out.reshape(B, C, 64, 64)



# revision 2
# speedup vs baseline: 1.0065x; 1.0065x over previous
"""Trainium2 Bass kernel for nn_Attention_57080115364834.

Reference computation (B=4, C=512, H=W=64, N=H*W=4096 tokens):
    t = x.reshape(b, c, n).swapaxes(1, 2)          # (b, n, c)
    q, k, v = t@Wq.T+bq, t@Wk.T+bk, t@Wv.T+bv
    attn = softmax(q @ k.T / sqrt(c))              # (b, n, n)
    out = (attn @ v) @ Wo.T + bo                   # (b, n, c)
    return out.reshape(b, c, h, w)                 # raw view, no permute

Sharding: 8 cores = 4 batches x 2 query-halves. Each core holds the full
x[b] (C x N, which is exactly t.T - the natural layout for Trainium
matmuls) so it computes its batch's full K^T (c,n) and VW (n,c) locally,
plus Q^T for its 2048-token half. No collectives.

Host-side algebra folds both post-attention linear steps away:
  - softmax rows sum to 1  =>  attn @ (v+bv) == attn@v + bv, so the v
    bias becomes an output bias  bo' = Wo @ bv + bo.
  - (attn @ v) @ Wo.T == attn @ (v @ Wo.T) == attn @ (t @ (Wo@Wv).T),
    so with Wvo = Wo@Wv precomputed on host, the VW projection directly
    produces final-channel values and no device-side output projection
    is needed.
The kernel returns outT (c, n) per core; the host transposes during
unsharding (a pure layout move).

Per-core dataflow (main matmuls bf16 with f32 PSUM accumulation; the
softmax normalization chain runs in f32/f32r, so 1/rowsum is exact):
  kT[c,m]   = Wk @ tC + bk   (lhsT=WkT chunk, rhs=tC chunk; bias on ACT evac)
  VW[m,c]   = tC.T @ WvoT    (lhsT=tC chunk,  rhs=WvoT)
  qT[c,n]   = Wq @ tCq + bq  per 512-token n-chunk
  ST[m,n]   = kT.T-chunks @ qT       (scores, transposed)
  P[m,n]    = exp(ST/sqrt(c))        ScalarE, no max-subtract (|scores|<~2)
  acc[m%128,n] += P                  DVE accumulate (for rowsum)
  OT[c,n]  += VW-chunk.T @ P         (PSUM-accumulated over m-tiles)
  otsb[c,n] = OT                     (DVE, PSUM->SBUF right away: frees the
                                      4 OT PSUM banks so the next chunk's
                                      attnv matmuls never wait on the
                                      normalization chain)
  rowsum[1,n] = ones.T @ acc (f32r MM); rinv broadcast via K=1 MM
  outT[c,n] = otsb * rinv_bc (DVE), + bo' via ACT bias-add -> DMA

The normalization tail for chunk nb is emitted inside chunk nb+1's
m-tile loop so the rowsum/reciprocal/broadcast chain is buried deep in
the PE instruction stream and never head-of-line-blocks the in-order
TensorEngine queue (which caused ~3.5us PE stalls + HAM re-throttles
per chunk in the v1 kernel).

A short warm-up burst of zero matmuls runs during the initial DMA
lead-in so the PE_HAM clock gate reaches 8/8 before real work arrives.
"""

import sys

for _p in ("/opt/trn_rl_repo", "/root/.axon_site/_ro/trn_rl_repo"):
    if _p not in sys.path:
        sys.path.append(_p)

import numpy as np
import ml_dtypes

import concourse.bacc as bacc
import concourse.mybir as mybir
import concourse.tile as tile
from concourse.bass_utils import run_bass_kernel_spmd

DT = mybir.dt.float32
FR = mybir.dt.float32r
BF = mybir.dt.bfloat16
AFT = mybir.ActivationFunctionType

B, C, HW = 4, 512, 4096          # batch, channels, tokens per batch
NQ = HW // 2                     # q tokens per core (2048)
CK = C // 128                    # contraction chunks (4)
MT = HW // 128                   # key/value tiles (32)
NB = NQ // 512                   # q-chunks per core (4)
SCALE = 1.0 / float(np.sqrt(C))
N_CORES = 8
N_WARM = 22                      # HAM warm-up matmuls

_compiled = None
_ONES = np.ones(128, dtype=np.float32)


def _build():
    nc = bacc.Bacc("TRN2", target_bir_lowering=False)

    xt_e = nc.declare_dram_parameter("xt", [C, HW], BF, isOutput=False)
    xq_e = nc.declare_dram_parameter("xq", [C, NQ], BF, isOutput=False)
    wqt_e = nc.declare_dram_parameter("wqt", [C, C], BF, isOutput=False)
    wkt_e = nc.declare_dram_parameter("wkt", [C, C], BF, isOutput=False)
    wvot_e = nc.declare_dram_parameter("wvot", [C, C], BF, isOutput=False)
    bq_e = nc.declare_dram_parameter("bq", [C], DT, isOutput=False)
    bk_e = nc.declare_dram_parameter("bk", [C], DT, isOutput=False)
    bop_e = nc.declare_dram_parameter("bop", [C], DT, isOutput=False)
    ones_fr_e = nc.declare_dram_parameter("ones_fr", [128], FR, isOutput=False)
    out_e = nc.declare_dram_parameter("outT", [C, NQ], DT, isOutput=True)

    with tile.TileContext(nc) as tc:
        # ---- HAM warm-up: zero matmuls with no DMA dependency keep the
        # PE busy through the initial DMA lead-in so the clock gate is at
        # 8/8 when real matmuls arrive. Pool closes -> PSUM bank reused.
        with (
            tc.tile_pool(name="warm", bufs=1) as warm_pool,
            tc.tile_pool(name="warmps", bufs=1, space="PSUM") as warm_ps,
        ):
            warm_sb = warm_pool.tile([128, 512], BF, tag="warm", name="warm_sb")
            nc.gpsimd.memset(warm_sb[:], 0.0)
            warm_ps_t = warm_ps.tile([128, 512], DT, tag="warmps", name="warm_ps")
            for i in range(N_WARM):
                nc.tensor.matmul(
                    warm_ps_t[:], warm_sb[:, 0:128], warm_sb[:],
                    start=(i == 0), stop=(i == N_WARM - 1),
                )

        with (
            tc.tile_pool(name="kt", bufs=1) as kt_pool,
            tc.tile_pool(name="vv", bufs=1) as vv_pool,
            tc.tile_pool(name="wq", bufs=1) as wq_pool,
            tc.tile_pool(name="consts", bufs=1) as c_pool,
        ):
            # ---- persistent tiles (phase-2-only DMAs emitted late so they
            # don't delay the first phase-1 matmul) ----
            kt_sb = [kt_pool.tile([128, HW], BF, tag=f"k{i}", name=f"k{i}") for i in range(CK)]
            vw_sb = [vv_pool.tile([128, C], BF, tag=f"v{i}", name=f"v{i}") for i in range(MT)]
            wq_sb = [wq_pool.tile([128, C], BF, tag=f"wq{i}", name=f"wq{i}") for i in range(CK)]

            bq_t = c_pool.tile([128, CK], DT, tag="bq", name="bq_t")
            bk_t = c_pool.tile([128, CK], DT, tag="bk", name="bk_t")
            bop_t = c_pool.tile([128, CK], DT, tag="bop", name="bop_t")
            ones_col_r = c_pool.tile([128, 1], FR, tag="onescr", name="ones_col_r")
            ones_row_r = c_pool.tile([1, 128], FR, tag="onesrr", name="ones_row_r")

            # ---- phase 1: kT (c,m) and VW (m,c) projections ----
            with (
                tc.tile_pool(name="wkv", bufs=1) as wkv_pool,
                tc.tile_pool(name="tcc", bufs=3) as tcc_pool,
                tc.tile_pool(name="ps1", bufs=2, space="PSUM") as ps1,
            ):
                wk_sb = [wkv_pool.tile([128, C], BF, tag=f"wk{i}", name=f"wk{i}") for i in range(CK)]
                wv_sb = [wkv_pool.tile([128, C], BF, tag=f"wv{i}", name=f"wv{i}") for i in range(CK)]

                # lead-in-critical DMAs first: j=0 x-tiles (sync queue, in
                # parallel with gpsimd's descriptors for later tiles) and
                # the Wk chunks feeding the first kT matmul group.
                tcs0 = [tcc_pool.tile([128, 512], BF, tag=f"tc{ci}", name=f"tc{ci}") for ci in range(CK)]
                for ci in range(CK):
                    nc.sync.dma_start(tcs0[ci][:], xt_e[ci * 128:(ci + 1) * 128, 0:512])
                for i in range(CK):
                    nc.sync.dma_start(wk_sb[i][:], wkt_e[i * 128:(i + 1) * 128, :])
                for t in range(CK):
                    nc.sync.dma_start(bk_t[:, t:t + 1], bk_e[t * 128:(t + 1) * 128])
                for i in range(CK):
                    nc.sync.dma_start(wv_sb[i][:], wvot_e[i * 128:(i + 1) * 128, :])
                nc.sync.dma_start(ones_col_r[:, 0:1], ones_fr_e[:])
                nc.sync.dma_start(ones_row_r[0:1, :], ones_fr_e[:])

                for j in range(HW // 512):
                    if j == 0:
                        tcs = tcs0
                    else:
                        tcs = [tcc_pool.tile([128, 512], BF, tag=f"tc{ci}", name=f"tc{ci}") for ci in range(CK)]
                        for ci in range(CK):
                            nc.gpsimd.dma_start(
                                tcs[ci][:], xt_e[ci * 128:(ci + 1) * 128, j * 512:(j + 1) * 512]
                            )
                    # kT token-chunk j, all four output-channel chunks
                    for co in range(CK):
                        pk = ps1.tile([128, 512], DT, tag="pk", name="pk")
                        for ci in range(CK):
                            nc.tensor.matmul(
                                pk[:], wk_sb[ci][:, co * 128:(co + 1) * 128],
                                tcs[ci][:], start=(ci == 0), stop=(ci == CK - 1),
                            )
                        nc.scalar.activation(
                            kt_sb[co][:, j * 512:(j + 1) * 512], pk[:], AFT.Identity,
                            bias=bk_t[:, co:co + 1],
                        )
                    # VW m-tiles 4j..4j+3 (no bias: folded into bo'),
                    # evacuation split DVE/ACT so neither engine's backlog
                    # delays the phase-1 -> phase-2 PSUM handover.
                    for ml in range(4):
                        pv = ps1.tile([128, 512], DT, tag="pv", name="pv")
                        for ci in range(CK):
                            nc.tensor.matmul(
                                pv[:], tcs[ci][:, ml * 128:(ml + 1) * 128],
                                wv_sb[ci][:], start=(ci == 0), stop=(ci == CK - 1),
                            )
                        if ml % 2 == 0:
                            nc.vector.tensor_copy(vw_sb[4 * j + ml][:], pv[:])
                        else:
                            nc.scalar.activation(vw_sb[4 * j + ml][:], pv[:], AFT.Copy)

            # phase-2 weights/consts arrive while phase-1 compute runs
            for i in range(CK):
                nc.sync.dma_start(wq_sb[i][:], wqt_e[i * 128:(i + 1) * 128, :])
            for t in range(CK):
                nc.sync.dma_start(bq_t[:, t:t + 1], bq_e[t * 128:(t + 1) * 128])
            for t in range(CK):
                nc.sync.dma_start(bop_t[:, t:t + 1], bop_e[t * 128:(t + 1) * 128])

            # ---- phase 2: attention per 512-token q-chunk ----
            with (
                tc.tile_pool(name="xqp", bufs=2) as xq_pool,
                tc.tile_pool(name="qcp", bufs=2) as qc_pool,
                tc.tile_pool(name="pexp", bufs=8) as pe_pool,
                tc.tile_pool(name="accp", bufs=2) as acc_pool,
                tc.tile_pool(name="rsp", bufs=2) as rs_pool,
                tc.tile_pool(name="otsbp", bufs=2) as otsb_pool,
                tc.tile_pool(name="outp", bufs=3) as out_pool,
                tc.tile_pool(name="smallp", bufs=2) as small_pool,
                tc.tile_pool(name="ps2", bufs=4, space="PSUM") as ps2,
                tc.tile_pool(name="psot", bufs=1, space="PSUM") as psot,
            ):
                def emit_tail(tnb, acc, otsb):
                    # rowsum via one f32r ones-matmul; reciprocal row;
                    # broadcast via K=1 matmul; normalize on DVE from the
                    # already-evacuated SBUF copy; exact bias via ACT add.
                    rs = ps2.tile([1, 512], DT, tag="st", name="rs")
                    nc.tensor.matmul(rs[:], ones_col_r[:, 0:1], acc[:], start=True, stop=True)
                    rinv_row = small_pool.tile([1, 512], FR, tag="rinvrow", name="rinv_row")
                    with nc.allow_low_precision(reason="f32r stores full f32 bits; PE rounds on read"):
                        nc.vector.reciprocal(rinv_row[:], rs[:])
                    rbc_ps = ps2.tile([128, 512], DT, tag="st", name="rbc_ps")
                    nc.tensor.matmul(rbc_ps[:], ones_row_r[0:1, :], rinv_row[0:1, :],
                                     start=True, stop=True)
                    rinv_bc = rs_pool.tile([128, 512], DT, tag="rinvbc", name="rinv_bc")
                    nc.vector.tensor_copy(rinv_bc[:], rbc_ps[:])
                    for co in range(CK):
                        om = out_pool.tile([128, 512], DT, tag="om", name="om", bufs=5)
                        nc.vector.tensor_mul(om[:], otsb[co][:], rinv_bc[:])
                        oc = out_pool.tile([128, 512], DT, tag="oc", name="oc", bufs=5)
                        nc.scalar.activation(oc[:], om[:], AFT.Identity,
                                             bias=bop_t[:, co:co + 1])
                        nc.sync.dma_start(
                            out_e[co * 128:(co + 1) * 128, tnb * 512:(tnb + 1) * 512], oc[:]
                        )

                prev = None
                for nb in range(NB):
                    xqs = [xq_pool.tile([128, 512], BF, tag=f"xq{ci}", name=f"xq{ci}") for ci in range(CK)]
                    for ci in range(CK):
                        nc.gpsimd.dma_start(
                            xqs[ci][:], xq_e[ci * 128:(ci + 1) * 128, nb * 512:(nb + 1) * 512]
                        )
                    # qT chunk (c, 512)
                    qcs = []
                    for co in range(CK):
                        pq = ps2.tile([128, 512], DT, tag="st", name="st")
                        for ci in range(CK):
                            nc.tensor.matmul(
                                pq[:], wq_sb[ci][:, co * 128:(co + 1) * 128],
                                xqs[ci][:], start=(ci == 0), stop=(ci == CK - 1),
                            )
                        qc = qc_pool.tile([128, 512], BF, tag=f"qc{co}", name=f"qc{co}")
                        nc.scalar.activation(qc[:], pq[:], AFT.Identity, bias=bq_t[:, co:co + 1])
                        qcs.append(qc)

                    acc = acc_pool.tile([128, 512], FR, tag="acc", name="acc")
                    ots = [psot.tile([128, 512], DT, tag=f"ot{co}", name=f"ot{co}") for co in range(CK)]
                    for mt in range(MT):
                        st = ps2.tile([128, 512], DT, tag="st", name="st")
                        for ci in range(CK):
                            nc.tensor.matmul(
                                st[:], kt_sb[ci][:, mt * 128:(mt + 1) * 128],
                                qcs[ci][:], start=(ci == 0), stop=(ci == CK - 1),
                            )
                        pexp = pe_pool.tile([128, 512], BF, tag="pe", name="pexp")
                        nc.scalar.activation(pexp[:], st[:], AFT.Exp, scale=SCALE)
                        if mt == 0:
                            nc.vector.tensor_copy(acc[:], pexp[:])
                        else:
                            nc.vector.tensor_add(acc[:], acc[:], pexp[:])
                        for co in range(CK):
                            nc.tensor.matmul(
                                ots[co][:], vw_sb[mt][:, co * 128:(co + 1) * 128],
                                pexp[:],
                                start=(mt == 0), stop=(mt == MT - 1), skip_group_check=True,
                            )
                        # previous chunk's normalization tail is emitted deep
                        # inside this chunk's m-loop: by then its inputs are
                        # long ready, so its PE ops never stall the queue
                        if mt == 3 and prev is not None:
                            emit_tail(*prev)

                    # immediately evacuate the (unnormalized) attention
                    # accumulators so the PSUM banks free up for chunk nb+1
                    otsb = []
                    for co in range(CK):
                        ob = otsb_pool.tile([128, 512], DT, tag=f"ob{co}", name=f"ob{co}")
                        nc.vector.tensor_copy(ob[:], ots[co][:])
                        otsb.append(ob)
                    prev = (nb, acc, otsb)

                emit_tail(*prev)

    nc.compile()
    return nc


def _get_compiled():
    global _compiled
    if _compiled is None:
        _compiled = _build()
    return _compiled


def kernel(**inputs):
    x = np.ascontiguousarray(np.asarray(inputs["x"], dtype=np.float32))
    wq = np.asarray(inputs["Wq"], dtype=np.float32)
    wk = np.asarray(inputs["Wk"], dtype=np.float32)
    wv = np.asarray(inputs["Wv"], dtype=np.float32)
    wo = np.asarray(inputs["Wo"], dtype=np.float32)
    bq = np.ascontiguousarray(np.asarray(inputs["bq"], dtype=np.float32))
    bk = np.ascontiguousarray(np.asarray(inputs["bk"], dtype=np.float32))
    bv = np.asarray(inputs["bv"], dtype=np.float32)
    bo = np.asarray(inputs["bo"], dtype=np.float32)

    wqt = np.ascontiguousarray(wq.T.astype(ml_dtypes.bfloat16))
    wkt = np.ascontiguousarray(wk.T.astype(ml_dtypes.bfloat16))
    wvot = np.ascontiguousarray((wo @ wv).T.astype(ml_dtypes.bfloat16))
    bop = np.ascontiguousarray(wo @ bv + bo)

    xb = x.reshape(B, C, HW).astype(ml_dtypes.bfloat16)
    in_maps = []
    for core in range(N_CORES):
        bi, h = core // 2, core % 2
        in_maps.append({
            "xt": np.ascontiguousarray(xb[bi]),
            "xq": np.ascontiguousarray(xb[bi][:, h * NQ:(h + 1) * NQ]),
            "wqt": wqt, "wkt": wkt, "wvot": wvot,
            "bq": bq, "bk": bk, "bop": bop, "ones_fr": _ONES,
        })

    nc = _get_compiled()
    res = run_bass_kernel_spmd(nc, in_maps, core_ids=list(range(N_CORES)))

    out = np.empty((B, HW, C), dtype=np.float32)
    for core in range(N_CORES):
        bi, h = core // 2, core % 2
        out[bi, h * NQ:(h + 1) * NQ, :] = res.results[core]["outT"].T
    return out.reshape(B, C, 64, 64)


# revision 3
# speedup vs baseline: 1.0518x; 1.0450x over previous
"""Trainium2 Bass kernel for nn_Attention_57080115364834.

Reference computation (B=4, C=512, H=W=64, N=H*W=4096 tokens):
    t = x.reshape(b, c, n).swapaxes(1, 2)          # (b, n, c)
    q, k, v = t@Wq.T+bq, t@Wk.T+bk, t@Wv.T+bv
    attn = softmax(q @ k.T / sqrt(c))              # (b, n, n)
    out = (attn @ v) @ Wo.T + bo                   # (b, n, c)
    return out.reshape(b, c, h, w)                 # raw view, no permute

Sharding: 8 cores = 4 batches x 2 query-halves. Each core holds the full
x[b] (C x N == t.T, the natural Trainium layout) so it computes its
batch's full K^T (c,n) and VW (n,c) locally, plus Q^T for its half.

Host-side algebra folds both post-attention linear steps away:
  - softmax rows sum to 1  =>  v bias becomes output bias bo' = Wo@bv+bo.
  - (attn@v)@Wo.T == attn@(t@(Wo@Wv).T), so with Wvo = Wo@Wv precomputed
    on host the VW projection directly produces final-channel values.
The kernel returns outT (c, n) per core; the host transposes while
unsharding (a pure layout move).

Per-core dataflow (matmuls bf16, f32 PSUM; normalization f32/f32r):
  kT[c,m]   = Wk @ tC + bk    VW[m,c] = tC.T @ WvoT     (phase 1)
  qT[c,n]   = Wq @ tCq + bq  per 512-token chunk (chunk 0 hoisted into
              phase 1 so the PSUM pool handover never idles the PE)
  ST[m,n]   = kT.T @ qT ; P = exp(ST/sqrt(c)) on ScalarE (no max-sub)
  acc      += P (DVE, for rowsum);  OT[c,n] += VW.T @ P  (PSUM-accum)
  otsb      = OT  (DVE PSUM->SBUF right away: frees the 4 OT banks so
              the next chunk's attnv matmuls never wait)
  rowsum = ones.T @ acc (f32r MM); rinv = 1/rowsum (DVE); broadcast via
  K=1 MM; outT = otsb*rinv_bc (DVE) + bo' (ACT bias) -> DMA

All engine queues are strict FIFO, so the normalization chain of chunk
nb is NOT emitted as one block: its ops are spread across chunk nb+1's
m-tile loop (rowsum@mt1, broadcast@mt4, one output channel at mt 6/8/
10/12).  Between any two chain ops every FIFO holds independent m-loop
work, so neither the PE nor the exp/acc pipeline ever stalls on the
chain (v1/v2 lost ~3.5us + a HAM re-throttle per chunk here).

A short warm-up burst of zero matmuls runs during the initial DMA
lead-in so the PE_HAM clock gate is already 8/8 when real work arrives.
"""

import sys

for _p in ("/opt/trn_rl_repo", "/root/.axon_site/_ro/trn_rl_repo"):
    if _p not in sys.path:
        sys.path.append(_p)

import numpy as np
import ml_dtypes

import concourse.bacc as bacc
import concourse.mybir as mybir
import concourse.tile as tile
from concourse.bass_utils import run_bass_kernel_spmd

DT = mybir.dt.float32
FR = mybir.dt.float32r
BF = mybir.dt.bfloat16
AFT = mybir.ActivationFunctionType

B, C, HW = 4, 512, 4096          # batch, channels, tokens per batch
NQ = HW // 2                     # q tokens per core (2048)
CK = C // 128                    # contraction chunks (4)
MT = HW // 128                   # key/value tiles (32)
NB = NQ // 512                   # q-chunks per core (4)
SCALE = 1.0 / float(np.sqrt(C))
N_CORES = 8
N_WARM = 12                      # HAM warm-up matmuls

_compiled = None
_ONES = np.ones(128, dtype=np.float32)


def _build():
    nc = bacc.Bacc("TRN2", target_bir_lowering=False)

    xt_e = nc.declare_dram_parameter("xt", [C, HW], BF, isOutput=False)
    xq_e = nc.declare_dram_parameter("xq", [C, NQ], BF, isOutput=False)
    wqt_e = nc.declare_dram_parameter("wqt", [C, C], BF, isOutput=False)
    wkt_e = nc.declare_dram_parameter("wkt", [C, C], BF, isOutput=False)
    wvot_e = nc.declare_dram_parameter("wvot", [C, C], BF, isOutput=False)
    bq_e = nc.declare_dram_parameter("bq", [C], DT, isOutput=False)
    bk_e = nc.declare_dram_parameter("bk", [C], DT, isOutput=False)
    bop_e = nc.declare_dram_parameter("bop", [C], DT, isOutput=False)
    ones_fr_e = nc.declare_dram_parameter("ones_fr", [128], FR, isOutput=False)
    out_e = nc.declare_dram_parameter("outT", [C, NQ], DT, isOutput=True)

    with tile.TileContext(nc) as tc:
        # ---- HAM warm-up: zero matmuls with no DMA dependency keep the
        # PE busy through the initial DMA lead-in so the clock gate is at
        # 8/8 when real matmuls arrive. Pool closes -> PSUM bank reused.
        with (
            tc.tile_pool(name="warm", bufs=1) as warm_pool,
            tc.tile_pool(name="warmps", bufs=1, space="PSUM") as warm_ps,
        ):
            warm_sb = warm_pool.tile([128, 512], BF, tag="warm", name="warm_sb")
            nc.gpsimd.memset(warm_sb[:], 0.0)
            warm_ps_t = warm_ps.tile([128, 512], DT, tag="warmps", name="warm_ps")
            for i in range(N_WARM):
                nc.tensor.matmul(
                    warm_ps_t[:], warm_sb[:, 0:128], warm_sb[:],
                    start=(i == 0), stop=(i == N_WARM - 1),
                )

        with (
            tc.tile_pool(name="kt", bufs=1) as kt_pool,
            tc.tile_pool(name="vv", bufs=1) as vv_pool,
            tc.tile_pool(name="wq", bufs=1) as wq_pool,
            tc.tile_pool(name="consts", bufs=1) as c_pool,
            tc.tile_pool(name="xqp", bufs=2) as xq_pool,
            tc.tile_pool(name="qcp", bufs=2) as qc_pool,
        ):
            # ---- persistent tiles ----
            kt_sb = [kt_pool.tile([128, HW], BF, tag=f"k{i}", name=f"k{i}") for i in range(CK)]
            vw_sb = [vv_pool.tile([128, C], BF, tag=f"v{i}", name=f"v{i}") for i in range(MT)]
            wq_sb = [wq_pool.tile([128, C], BF, tag=f"wq{i}", name=f"wq{i}") for i in range(CK)]

            bq_t = c_pool.tile([128, CK], DT, tag="bq", name="bq_t")
            bk_t = c_pool.tile([128, CK], DT, tag="bk", name="bk_t")
            bop_t = c_pool.tile([128, CK], DT, tag="bop", name="bop_t")
            ones_col_r = c_pool.tile([128, 1], FR, tag="onescr", name="ones_col_r")
            ones_row_r = c_pool.tile([1, 128], FR, tag="onesrr", name="ones_row_r")

            def project_q(nb, pool, ps_tag_pool, ps_tag, ps_bufs):
                xqs = [xq_pool.tile([128, 512], BF, tag=f"xq{ci}", name=f"xq{ci}") for ci in range(CK)]
                for ci in range(CK):
                    nc.gpsimd.dma_start(
                        xqs[ci][:], xq_e[ci * 128:(ci + 1) * 128, nb * 512:(nb + 1) * 512]
                    )
                qcs = []
                for co in range(CK):
                    pq = ps_tag_pool.tile([128, 512], DT, tag=ps_tag, name="pq", bufs=ps_bufs)
                    for ci in range(CK):
                        nc.tensor.matmul(
                            pq[:], wq_sb[ci][:, co * 128:(co + 1) * 128],
                            xqs[ci][:], start=(ci == 0), stop=(ci == CK - 1),
                        )
                    qc = qc_pool.tile([128, 512], BF, tag=f"qc{co}", name=f"qc{co}")
                    nc.scalar.activation(qc[:], pq[:], AFT.Identity, bias=bq_t[:, co:co + 1])
                    qcs.append(qc)
                return qcs

            # ---- phase 1: kT (c,m) and VW (m,c) projections ----
            with (
                tc.tile_pool(name="wkv", bufs=1) as wkv_pool,
                tc.tile_pool(name="tcc", bufs=3) as tcc_pool,
                tc.tile_pool(name="ps1", bufs=2, space="PSUM") as ps1,
            ):
                wk_sb = [wkv_pool.tile([128, C], BF, tag=f"wk{i}", name=f"wk{i}") for i in range(CK)]
                wv_sb = [wkv_pool.tile([128, C], BF, tag=f"wv{i}", name=f"wv{i}") for i in range(CK)]

                # lead-in-critical DMAs first: j=0/j=1 x-tiles (sync queue,
                # in parallel with gpsimd's descriptors for later tiles) and
                # the Wk chunks feeding the first kT matmul group.
                tcs01 = [tcc_pool.tile([128, 512], BF, tag=f"tc{ci}", name=f"tc{ci}") for ci in range(CK)]
                for ci in range(CK):
                    nc.sync.dma_start(tcs01[ci][:], xt_e[ci * 128:(ci + 1) * 128, 0:512])
                for i in range(CK):
                    nc.sync.dma_start(wk_sb[i][:], wkt_e[i * 128:(i + 1) * 128, :])
                for t in range(CK):
                    nc.sync.dma_start(bk_t[:, t:t + 1], bk_e[t * 128:(t + 1) * 128])
                for i in range(CK):
                    nc.sync.dma_start(wv_sb[i][:], wvot_e[i * 128:(i + 1) * 128, :])
                nc.sync.dma_start(ones_col_r[:, 0:1], ones_fr_e[:])
                nc.sync.dma_start(ones_row_r[0:1, :], ones_fr_e[:])
                # phase-2 weights early on the sync queue as well
                for i in range(CK):
                    nc.sync.dma_start(wq_sb[i][:], wqt_e[i * 128:(i + 1) * 128, :])
                for t in range(CK):
                    nc.sync.dma_start(bq_t[:, t:t + 1], bq_e[t * 128:(t + 1) * 128])
                for t in range(CK):
                    nc.sync.dma_start(bop_t[:, t:t + 1], bop_e[t * 128:(t + 1) * 128])

                qcs0 = None
                for j in range(HW // 512):
                    if j == 0:
                        tcs = tcs01
                    else:
                        tcs = [tcc_pool.tile([128, 512], BF, tag=f"tc{ci}", name=f"tc{ci}") for ci in range(CK)]
                        for ci in range(CK):
                            (nc.sync if j == 1 else nc.gpsimd).dma_start(
                                tcs[ci][:], xt_e[ci * 128:(ci + 1) * 128, j * 512:(j + 1) * 512]
                            )
                    # kT token-chunk j, all four output-channel chunks
                    for co in range(CK):
                        pk = ps1.tile([128, 512], DT, tag="pk", name="pk")
                        for ci in range(CK):
                            nc.tensor.matmul(
                                pk[:], wk_sb[ci][:, co * 128:(co + 1) * 128],
                                tcs[ci][:], start=(ci == 0), stop=(ci == CK - 1),
                            )
                        nc.scalar.activation(
                            kt_sb[co][:, j * 512:(j + 1) * 512], pk[:], AFT.Identity,
                            bias=bk_t[:, co:co + 1],
                        )
                    # VW m-tiles 4j..4j+3 (no bias: folded into bo'),
                    # evacuation split DVE/ACT to balance engine backlogs.
                    for ml in range(4):
                        pv = ps1.tile([128, 512], DT, tag="pv", name="pv")
                        for ci in range(CK):
                            nc.tensor.matmul(
                                pv[:], tcs[ci][:, ml * 128:(ml + 1) * 128],
                                wv_sb[ci][:], start=(ci == 0), stop=(ci == CK - 1),
                            )
                        if ml % 2 == 0:
                            nc.vector.tensor_copy(vw_sb[4 * j + ml][:], pv[:])
                        else:
                            nc.scalar.activation(vw_sb[4 * j + ml][:], pv[:], AFT.Copy)
                    # chunk-0 q projection rides inside phase 1 so phase 2
                    # can open with score matmuls immediately
                    if j == 4:
                        qcs0 = project_q(0, None, ps1, "pq", 1)

            # ---- phase 2: attention per 512-token q-chunk ----
            with (
                tc.tile_pool(name="pexp", bufs=8) as pe_pool,
                tc.tile_pool(name="accp", bufs=2) as acc_pool,
                tc.tile_pool(name="rsp", bufs=2) as rs_pool,
                tc.tile_pool(name="otsbp", bufs=2) as otsb_pool,
                tc.tile_pool(name="outp", bufs=3) as out_pool,
                tc.tile_pool(name="smallp", bufs=2) as small_pool,
                tc.tile_pool(name="ps2", bufs=4, space="PSUM") as ps2,
                tc.tile_pool(name="psot", bufs=1, space="PSUM") as psot,
            ):
                def tail_rowsum(tnb, acc, otsb):
                    rs = ps2.tile([1, 512], DT, tag="st", name="rs")
                    nc.tensor.matmul(rs[:], ones_col_r[:, 0:1], acc[:], start=True, stop=True)
                    rinv_row = small_pool.tile([1, 512], FR, tag="rinvrow", name="rinv_row")
                    with nc.allow_low_precision(reason="f32r stores full f32 bits; PE rounds on read"):
                        nc.vector.reciprocal(rinv_row[:], rs[:])
                    return rinv_row

                def tail_bcast(rinv_row):
                    rbc_ps = ps2.tile([128, 512], DT, tag="st", name="rbc_ps")
                    nc.tensor.matmul(rbc_ps[:], ones_row_r[0:1, :], rinv_row[0:1, :],
                                     start=True, stop=True)
                    rinv_bc = rs_pool.tile([128, 512], DT, tag="rinvbc", name="rinv_bc")
                    nc.vector.tensor_copy(rinv_bc[:], rbc_ps[:])
                    return rinv_bc

                def tail_out(tnb, co, src, rinv_bc):
                    om = out_pool.tile([128, 512], DT, tag="om", name="om", bufs=5)
                    nc.vector.tensor_mul(om[:], src[:], rinv_bc[:])
                    oc = out_pool.tile([128, 512], DT, tag="oc", name="oc", bufs=5)
                    nc.scalar.activation(oc[:], om[:], AFT.Identity,
                                         bias=bop_t[:, co:co + 1])
                    nc.sync.dma_start(
                        out_e[co * 128:(co + 1) * 128, tnb * 512:(tnb + 1) * 512], oc[:]
                    )

                prev = None
                tail_state = {}
                for nb in range(NB):
                    qcs = qcs0 if nb == 0 else project_q(nb, None, ps2, "st", None)

                    acc = acc_pool.tile([128, 512], FR, tag="acc", name="acc")
                    ots = [psot.tile([128, 512], DT, tag=f"ot{co}", name=f"ot{co}") for co in range(CK)]
                    for mt in range(MT):
                        st = ps2.tile([128, 512], DT, tag="st", name="st")
                        for ci in range(CK):
                            nc.tensor.matmul(
                                st[:], kt_sb[ci][:, mt * 128:(mt + 1) * 128],
                                qcs[ci][:], start=(ci == 0), stop=(ci == CK - 1),
                            )
                        pexp = pe_pool.tile([128, 512], BF, tag="pe", name="pexp")
                        nc.scalar.activation(pexp[:], st[:], AFT.Exp, scale=SCALE)
                        if mt == 0:
                            nc.vector.tensor_copy(acc[:], pexp[:])
                        else:
                            nc.vector.tensor_add(acc[:], acc[:], pexp[:])
                        for co in range(CK):
                            nc.tensor.matmul(
                                ots[co][:], vw_sb[mt][:, co * 128:(co + 1) * 128],
                                pexp[:],
                                start=(mt == 0), stop=(mt == MT - 1), skip_group_check=True,
                            )
                        # the previous chunk's normalization chain, spread
                        # one op per emission point so no FIFO ever backs up
                        if prev is not None:
                            if mt == 1:
                                tail_state["rinv_row"] = tail_rowsum(*prev)
                            elif mt == 4:
                                tail_state["rinv_bc"] = tail_bcast(tail_state["rinv_row"])
                            elif mt in (6, 8, 10, 12):
                                co = (mt - 6) // 2
                                tail_out(prev[0], co, prev[2][co], tail_state["rinv_bc"])

                    if nb < NB - 1:
                        # evacuate unnormalized accumulators -> frees the 4
                        # OT PSUM banks for the next chunk immediately
                        otsb = []
                        for co in range(CK):
                            ob = otsb_pool.tile([128, 512], DT, tag=f"ob{co}", name=f"ob{co}")
                            nc.vector.tensor_copy(ob[:], ots[co][:])
                            otsb.append(ob)
                        prev = (nb, acc, otsb)
                    else:
                        # last chunk: normalize straight out of PSUM
                        rinv_row = tail_rowsum(nb, acc, None)
                        rinv_bc = tail_bcast(rinv_row)
                        for co in range(CK):
                            tail_out(nb, co, ots[co], rinv_bc)

    nc.compile()
    return nc


def _get_compiled():
    global _compiled
    if _compiled is None:
        _compiled = _build()
    return _compiled


def kernel(**inputs):
    x = np.ascontiguousarray(np.asarray(inputs["x"], dtype=np.float32))
    wq = np.asarray(inputs["Wq"], dtype=np.float32)
    wk = np.asarray(inputs["Wk"], dtype=np.float32)
    wv = np.asarray(inputs["Wv"], dtype=np.float32)
    wo = np.asarray(inputs["Wo"], dtype=np.float32)
    bq = np.ascontiguousarray(np.asarray(inputs["bq"], dtype=np.float32))
    bk = np.ascontiguousarray(np.asarray(inputs["bk"], dtype=np.float32))
    bv = np.asarray(inputs["bv"], dtype=np.float32)
    bo = np.asarray(inputs["bo"], dtype=np.float32)

    wqt = np.ascontiguousarray(wq.T.astype(ml_dtypes.bfloat16))
    wkt = np.ascontiguousarray(wk.T.astype(ml_dtypes.bfloat16))
    wvot = np.ascontiguousarray((wo @ wv).T.astype(ml_dtypes.bfloat16))
    bop = np.ascontiguousarray(wo @ bv + bo)

    xb = x.reshape(B, C, HW).astype(ml_dtypes.bfloat16)
    in_maps = []
    for core in range(N_CORES):
        bi, h = core // 2, core % 2
        in_maps.append({
            "xt": np.ascontiguousarray(xb[bi]),
            "xq": np.ascontiguousarray(xb[bi][:, h * NQ:(h + 1) * NQ]),
            "wqt": wqt, "wkt": wkt, "wvot": wvot,
            "bq": bq, "bk": bk, "bop": bop, "ones_fr": _ONES,
        })

    nc = _get_compiled()
    res = run_bass_kernel_spmd(nc, in_maps, core_ids=list(range(N_CORES)))

    out = np.empty((B, HW, C), dtype=np.float32)
    for core in range(N_CORES):
        bi, h = core // 2, core % 2
        out[bi, h * NQ:(h + 1) * NQ, :] = res.results[core]["outT"].T
    return out.reshape(B, C, 64, 64)


# revision 11
# speedup vs baseline: 1.0708x; 1.0180x over previous
"""Trainium2 Bass kernel for nn_Attention_57080115364834.

Reference computation (B=4, C=512, H=W=64, N=H*W=4096 tokens):
    t = x.reshape(b, c, n).swapaxes(1, 2)          # (b, n, c)
    q, k, v = t@Wq.T+bq, t@Wk.T+bk, t@Wv.T+bv
    attn = softmax(q @ k.T / sqrt(c))              # (b, n, n)
    out = (attn @ v) @ Wo.T + bo                   # (b, n, c)
    return out.reshape(b, c, h, w)                 # raw view, no permute

Sharding: 8 cores = 4 batches x 2 query-halves. Each core holds the full
x[b] (C x N == t.T, the natural Trainium layout) so it computes its
batch's full K^T (c,n) and VW (n,c) locally, plus Q^T for its half.

Host-side algebra folds both post-attention linear steps away:
  - softmax rows sum to 1  =>  v bias becomes output bias bo' = Wo@bv+bo,
    applied on the host after gathering (a per-channel constant add).
  - (attn@v)@Wo.T == attn@(t@(Wo@Wv).T), so with Wvo = Wo@Wv precomputed
    on host the VW projection directly produces final-channel values.

Per-core dataflow (matmuls bf16, f32 PSUM; normalization f32/f32r):
  kT[c,m]   = Wk @ tC + bk    VW[m,c] = tC.T @ WvoT     (phase 1)
  qT[c,n]   = Wq @ tCq + bq  per 512-token chunk (chunk 0 hoisted into
              phase 1 so the PSUM pool handover never idles the PE)
  ST[m,n]   = kT.T @ qT ; P = exp(ST/sqrt(c)) on ScalarE (no max-sub)
  acc      += P (DVE, f32r, for the rowsums)
  OT[n,c]  += P-chunk.T @ VW          (PSUM-accum over m-tiles; output is
              token-major, so rowsums live one-per-partition)
  otsb      = OT   (PSUM->SBUF split DVE/ACT; frees the 4 OT banks so the
              next chunk's attnv matmuls never wait)
  rs[tl]    = acc-chunk.T @ ones      (4 tiny N=1 matmuls -> [128,4])
  out[n,c]  = otsb / rs  via GPSIMD normalize_recip (per-partition denom)
              -> DMA, token-major, no transpose needed anywhere.

The old channel-major design needed reciprocal+broadcast of a [1,512]
row: a single-lane 3.3us DVE reciprocal plus a K=1 broadcast matmul sat
in the serial chain of every chunk and fully exposed at kernel end.
Token-major output turns that into per-partition work on the otherwise
idle GPSIMD engine.

Engine queues are strict FIFO, so chunk nb's normalization ops are
spread across chunk nb+1 (rowsums after qproj, normalize+DMA at mt==2)
so no FIFO ever backs up behind a dependency chain.

A short warm-up burst of zero matmuls runs during the initial DMA
lead-in so the PE_HAM clock gate is already 8/8 when real work arrives.
"""

import sys

for _p in ("/opt/trn_rl_repo", "/root/.axon_site/_ro/trn_rl_repo"):
    if _p not in sys.path:
        sys.path.append(_p)

import numpy as np
import ml_dtypes

import concourse.bacc as bacc
import concourse.mybir as mybir
import concourse.tile as tile
from concourse.bass_utils import run_bass_kernel_spmd

DT = mybir.dt.float32
FR = mybir.dt.float32r
BF = mybir.dt.bfloat16
AFT = mybir.ActivationFunctionType

B, C, HW = 4, 512, 4096          # batch, channels, tokens per batch
NQ = HW // 2                     # q tokens per core (2048)
CK = C // 128                    # contraction chunks (4)
MT = HW // 128                   # key/value tiles (32)
NB = NQ // 512                   # q-chunks per core (4)
SCALE = 1.0 / float(np.sqrt(C))
N_CORES = 8
N_WARM = 12                      # HAM warm-up matmuls

_compiled = None
_ONES_BF = np.ones(128, dtype=ml_dtypes.bfloat16)


def _build():
    nc = bacc.Bacc("TRN2", target_bir_lowering=False)

    xt_e = nc.declare_dram_parameter("xt", [C, HW], BF, isOutput=False)
    xq_e = nc.declare_dram_parameter("xq", [C, NQ], BF, isOutput=False)
    wqt_e = nc.declare_dram_parameter("wqt", [C, C], BF, isOutput=False)
    wkt_e = nc.declare_dram_parameter("wkt", [C, C], BF, isOutput=False)
    wvot_e = nc.declare_dram_parameter("wvot", [C, C], BF, isOutput=False)
    bq_e = nc.declare_dram_parameter("bq", [C], DT, isOutput=False)
    bk_e = nc.declare_dram_parameter("bk", [C], DT, isOutput=False)
    ones_bf_e = nc.declare_dram_parameter("ones_bf", [128], BF, isOutput=False)
    out_e = nc.declare_dram_parameter("out", [NQ, C], DT, isOutput=True)

    with tile.TileContext(nc) as tc:
        # ---- HAM warm-up: zero matmuls with no DMA dependency keep the
        # PE busy through the initial DMA lead-in so the clock gate is at
        # 8/8 when real matmuls arrive. Pool closes -> PSUM bank reused.
        with (
            tc.tile_pool(name="warm", bufs=1) as warm_pool,
            tc.tile_pool(name="warmps", bufs=1, space="PSUM") as warm_ps,
        ):
            warm_sb = warm_pool.tile([128, 512], BF, tag="warm", name="warm_sb")
            nc.gpsimd.memset(warm_sb[:], 0.0)
            warm_ps_t = warm_ps.tile([128, 512], DT, tag="warmps", name="warm_ps")
            for i in range(N_WARM):
                nc.tensor.matmul(
                    warm_ps_t[:], warm_sb[:, 0:128], warm_sb[:],
                    start=(i == 0), stop=(i == N_WARM - 1),
                )

        with (
            tc.tile_pool(name="kt", bufs=1) as kt_pool,
            tc.tile_pool(name="vv", bufs=1) as vv_pool,
            tc.tile_pool(name="wq", bufs=1) as wq_pool,
            tc.tile_pool(name="consts", bufs=1) as c_pool,
            tc.tile_pool(name="xqp", bufs=2) as xq_pool,
            tc.tile_pool(name="qcp", bufs=2) as qc_pool,
        ):
            # ---- persistent tiles ----
            kt_sb = [kt_pool.tile([128, HW], BF, tag=f"k{i}", name=f"k{i}") for i in range(CK)]
            vw_sb = [vv_pool.tile([128, C], BF, tag=f"v{i}", name=f"v{i}") for i in range(MT)]
            wq_sb = [wq_pool.tile([128, C], BF, tag=f"wq{i}", name=f"wq{i}") for i in range(CK)]

            bq_t = c_pool.tile([128, CK], DT, tag="bq", name="bq_t")
            bk_t = c_pool.tile([128, CK], DT, tag="bk", name="bk_t")
            ones_col_b = c_pool.tile([128, 1], BF, tag="onescb", name="ones_col_b")

            def project_q(nb, ps_pool, ps_tag, ps_bufs):
                xqs = [xq_pool.tile([128, 512], BF, tag=f"xq{ci}", name=f"xq{ci}") for ci in range(CK)]
                for ci in range(CK):
                    nc.gpsimd.dma_start(
                        xqs[ci][:], xq_e[ci * 128:(ci + 1) * 128, nb * 512:(nb + 1) * 512]
                    )
                qcs = []
                for co in range(CK):
                    pq = ps_pool.tile([128, 512], DT, tag=ps_tag, name="pq", bufs=ps_bufs)
                    for ci in range(CK):
                        nc.tensor.matmul(
                            pq[:], wq_sb[ci][:, co * 128:(co + 1) * 128],
                            xqs[ci][:], start=(ci == 0), stop=(ci == CK - 1),
                        )
                    qc = qc_pool.tile([128, 512], BF, tag=f"qc{co}", name=f"qc{co}")
                    nc.scalar.activation(qc[:], pq[:], AFT.Identity, bias=bq_t[:, co:co + 1])
                    qcs.append(qc)
                return qcs

            # ---- phase 1: kT (c,m) and VW (m,c) projections ----
            with (
                tc.tile_pool(name="wkv", bufs=1) as wkv_pool,
                tc.tile_pool(name="tcc", bufs=3) as tcc_pool,
                tc.tile_pool(name="ps1", bufs=2, space="PSUM") as ps1,
            ):
                wk_sb = [wkv_pool.tile([128, C], BF, tag=f"wk{i}", name=f"wk{i}") for i in range(CK)]
                wv_sb = [wkv_pool.tile([128, C], BF, tag=f"wv{i}", name=f"wv{i}") for i in range(CK)]

                # lead-in-critical DMAs first, in consumption order
                tcs01 = [tcc_pool.tile([128, 512], BF, tag=f"tc{ci}", name=f"tc{ci}") for ci in range(CK)]
                for ci in range(CK):
                    nc.sync.dma_start(tcs01[ci][:], xt_e[ci * 128:(ci + 1) * 128, 0:512])
                for i in range(CK):
                    nc.sync.dma_start(wk_sb[i][:], wkt_e[i * 128:(i + 1) * 128, :])
                for i in range(CK):
                    nc.sync.dma_start(wv_sb[i][:], wvot_e[i * 128:(i + 1) * 128, :])
                tcs1 = [tcc_pool.tile([128, 512], BF, tag=f"tc{ci}", name=f"tc{ci}") for ci in range(CK)]
                for ci in range(CK):
                    nc.sync.dma_start(tcs1[ci][:], xt_e[ci * 128:(ci + 1) * 128, 512:1024])
                for t in range(CK):
                    nc.sync.dma_start(bk_t[:, t:t + 1], bk_e[t * 128:(t + 1) * 128])
                nc.sync.dma_start(ones_col_b[:, 0:1], ones_bf_e[:])
                for i in range(CK):
                    nc.sync.dma_start(wq_sb[i][:], wqt_e[i * 128:(i + 1) * 128, :])
                for t in range(CK):
                    nc.sync.dma_start(bq_t[:, t:t + 1], bq_e[t * 128:(t + 1) * 128])

                qcs0 = None
                for j in range(HW // 512):
                    if j == 0:
                        tcs = tcs01
                    elif j == 1:
                        tcs = tcs1
                    else:
                        tcs = [tcc_pool.tile([128, 512], BF, tag=f"tc{ci}", name=f"tc{ci}") for ci in range(CK)]
                        for ci in range(CK):
                            nc.gpsimd.dma_start(
                                tcs[ci][:], xt_e[ci * 128:(ci + 1) * 128, j * 512:(j + 1) * 512]
                            )
                    # kT token-chunk j, all four output-channel chunks
                    for co in range(CK):
                        pk = ps1.tile([128, 512], DT, tag="pk", name="pk")
                        for ci in range(CK):
                            nc.tensor.matmul(
                                pk[:], wk_sb[ci][:, co * 128:(co + 1) * 128],
                                tcs[ci][:], start=(ci == 0), stop=(ci == CK - 1),
                            )
                        nc.scalar.activation(
                            kt_sb[co][:, j * 512:(j + 1) * 512], pk[:], AFT.Identity,
                            bias=bk_t[:, co:co + 1],
                        )
                    # VW m-tiles 4j..4j+3 (no bias: folded into bo'),
                    # evacuation split DVE/ACT to balance engine backlogs.
                    for ml in range(4):
                        pv = ps1.tile([128, 512], DT, tag="pv", name="pv")
                        for ci in range(CK):
                            nc.tensor.matmul(
                                pv[:], tcs[ci][:, ml * 128:(ml + 1) * 128],
                                wv_sb[ci][:], start=(ci == 0), stop=(ci == CK - 1),
                            )
                        if ml % 2 == 0:
                            nc.vector.tensor_copy(vw_sb[4 * j + ml][:], pv[:])
                        else:
                            nc.scalar.activation(vw_sb[4 * j + ml][:], pv[:], AFT.Copy)
                    # chunk-0 q projection rides inside phase 1 so phase 2
                    # can open with score matmuls immediately
                    if j == 4:
                        qcs0 = project_q(0, ps1, "pq", 1)

            # ---- phase 2: attention per 512-token q-chunk ----
            with (
                tc.tile_pool(name="pexp", bufs=8) as pe_pool,
                tc.tile_pool(name="accp", bufs=2) as acc_pool,
                tc.tile_pool(name="otsbp", bufs=2) as otsb_pool,
                tc.tile_pool(name="outp", bufs=3) as out_pool,
                tc.tile_pool(name="smallp", bufs=2) as small_pool,
                tc.tile_pool(name="ps2", bufs=4, space="PSUM") as ps2,
                tc.tile_pool(name="psot", bufs=1, space="PSUM") as psot,
            ):
                def tail_rowsums(acc_bf):
                    # transposed rowsums: [128 tokens, 1] per 128-token group,
                    # so the denominators land one-per-partition (fp32r can't
                    # be a wide stationary operand, hence the bf16 acc copy;
                    # the 128-way sum averages the rounding noise to ~4e-4)
                    rs_ps = ps2.tile([128, CK], DT, tag="st", name="rs_ps")
                    for tl in range(CK):
                        nc.tensor.matmul(
                            rs_ps[:, tl:tl + 1], acc_bf[:, tl * 128:(tl + 1) * 128],
                            ones_col_b[:, 0:1],
                            start=(tl == 0), stop=(tl == CK - 1),
                            skip_group_check=(tl > 0),
                        )
                    rs_sb = small_pool.tile([128, CK], DT, tag="rssb", name="rs_sb")
                    nc.vector.tensor_copy(rs_sb[:], rs_ps[:])
                    return rs_sb

                def tail_norm_out(tnb, tl, otsb_t, rs_sb):
                    fin = out_pool.tile([128, C], DT, tag="fin", name="fin", bufs=5)
                    nc.gpsimd.normalize_recip(fin[:], otsb_t[:], rs_sb[:, tl:tl + 1])
                    nc.gpsimd.dma_start(
                        out_e[tnb * 512 + tl * 128:tnb * 512 + (tl + 1) * 128, :], fin[:]
                    )

                prev = None
                for nb in range(NB):
                    qcs = qcs0 if nb == 0 else project_q(nb, ps2, "st", None)
                    if prev is not None:
                        prev = (*prev, tail_rowsums(prev[1]))

                    acc = acc_pool.tile([128, 512], FR, tag="acc", name="acc")
                    ots = [psot.tile([128, C], DT, tag=f"ot{tl}", name=f"ot{tl}") for tl in range(CK)]
                    for mt in range(MT):
                        st = ps2.tile([128, 512], DT, tag="st", name="st")
                        for ci in range(CK):
                            nc.tensor.matmul(
                                st[:], kt_sb[ci][:, mt * 128:(mt + 1) * 128],
                                qcs[ci][:], start=(ci == 0), stop=(ci == CK - 1),
                            )
                        pexp = pe_pool.tile([128, 512], BF, tag="pe", name="pexp")
                        nc.scalar.activation(pexp[:], st[:], AFT.Exp, scale=SCALE)
                        if mt == 0:
                            nc.vector.tensor_copy(acc[:], pexp[:])
                        else:
                            nc.vector.tensor_add(acc[:], acc[:], pexp[:])
                        for tl in range(CK):
                            nc.tensor.matmul(
                                ots[tl][:], pexp[:, tl * 128:(tl + 1) * 128],
                                vw_sb[mt][:],
                                start=(mt == 0), stop=(mt == MT - 1), skip_group_check=True,
                            )
                        if mt == 2 and prev is not None:
                            tnb, _, otsb, rs_sb = prev
                            for tl in range(CK):
                                tail_norm_out(tnb, tl, otsb[tl], rs_sb)

                    # evacuate unnormalized accumulators (DVE/ACT split) ->
                    # frees the 4 OT PSUM banks for the next chunk at once
                    acc_bf = acc_pool.tile([128, 512], BF, tag="accbf", name="acc_bf")
                    nc.vector.tensor_copy(acc_bf[:], acc[:])
                    otsb = []
                    for tl in range(CK):
                        ob = otsb_pool.tile([128, C], DT, tag=f"ob{tl}", name=f"ob{tl}")
                        if tl % 2 == 0:
                            nc.vector.tensor_copy(ob[:], ots[tl][:])
                        else:
                            nc.scalar.activation(ob[:], ots[tl][:], AFT.Copy)
                        otsb.append(ob)
                    prev = (nb, acc_bf, otsb)

                # last chunk: emit its whole tail immediately
                tnb, acc_bf, otsb = prev
                rs_sb = tail_rowsums(acc_bf)
                for tl in range(CK):
                    tail_norm_out(tnb, tl, otsb[tl], rs_sb)

    nc.compile()
    return nc


def _get_compiled():
    global _compiled
    if _compiled is None:
        _compiled = _build()
    return _compiled


def kernel(**inputs):
    x = np.ascontiguousarray(np.asarray(inputs["x"], dtype=np.float32))
    wq = np.asarray(inputs["Wq"], dtype=np.float32)
    wk = np.asarray(inputs["Wk"], dtype=np.float32)
    wv = np.asarray(inputs["Wv"], dtype=np.float32)
    wo = np.asarray(inputs["Wo"], dtype=np.float32)
    bq = np.ascontiguousarray(np.asarray(inputs["bq"], dtype=np.float32))
    bk = np.ascontiguousarray(np.asarray(inputs["bk"], dtype=np.float32))
    bv = np.asarray(inputs["bv"], dtype=np.float32)
    bo = np.asarray(inputs["bo"], dtype=np.float32)

    wqt = np.ascontiguousarray(wq.T.astype(ml_dtypes.bfloat16))
    wkt = np.ascontiguousarray(wk.T.astype(ml_dtypes.bfloat16))
    wvot = np.ascontiguousarray((wo @ wv).T.astype(ml_dtypes.bfloat16))
    bop = wo @ bv + bo

    xb = x.reshape(B, C, HW).astype(ml_dtypes.bfloat16)
    in_maps = []
    for core in range(N_CORES):
        bi, h = core // 2, core % 2
        in_maps.append({
            "xt": np.ascontiguousarray(xb[bi]),
            "xq": np.ascontiguousarray(xb[bi][:, h * NQ:(h + 1) * NQ]),
            "wqt": wqt, "wkt": wkt, "wvot": wvot,
            "bq": bq, "bk": bk, "ones_bf": _ONES_BF,
        })

    nc = _get_compiled()
    res = run_bass_kernel_spmd(nc, in_maps, core_ids=list(range(N_CORES)))

    out = np.empty((B, HW, C), dtype=np.float32)
    for core in range(N_CORES):
        bi, h = core // 2, core % 2
        out[bi, h * NQ:(h + 1) * NQ, :] = res.results[core]["out"]
    out += bop  # bo' = Wo@bv + bo, exact because softmax rows sum to 1
    return out.reshape(B, C, 64, 64)
